# revision 27
# baseline (speedup 1.0000x reference)
"""Causal multi-head attention (B=2, S=2048, D=1024, H=16) on 8 NeuronCores.

Sharding: head-parallel. Core c owns heads {2c, 2c+1} = a 128-wide slice of
the q/k/v projection output dims and of wo's input dim. Each core computes
attention for its 2 heads over both batch elements and a full-size partial
of the final projection; the host sums the 8 partials.

v2 (all-bf16): inputs, weights, internal operand tiles, and the output
partials are bf16 (fp32 PSUM accumulation everywhere), halving HBM traffic
and enabling full-rate PE on every tile size. V is projected directly in
transposed layout (stationary = x-block, moving = wv), eliminating the PE
transpose pass. Scores are computed transposed (scoresT[k, q]) so softmax
probs feed attn@v without transposition; a ones-column in vN makes the same
matmul emit softmax denominators. |scores/8| < ~3 so exp without
max-subtraction is exact. DMAs are batched (1 load/chunk, 2-block stores)
and issued from both SP and ACT queues to avoid sequencer serialization.
Engine balance: PE does matmuls only; ACT does exp + half the output-stage
copies; DVE does qk/v copies, reciprocal + normalize; Pool (gpsimd) does
diag masks, denominator broadcast, and the other output-stage copies.
"""
import numpy as np
import ml_dtypes

import concourse.bass as bass
import concourse.tile as tile
from concourse import bacc, mybir
from concourse.bass_utils import run_bass_kernel_spmd

B, S, D = 2, 2048, 1024
H, HD = 16, 64
NCORES = 8
SF = B * S              # 4096 flattened rows
CH = 512                # column chunk for matmuls
KT = 128                # k-tile (keys per tile)
NEG = -1.0e38

F32 = mybir.dt.float32
BF16 = mybir.dt.bfloat16

_cache = {}


def _emit_body(nc, tc, io, rep):
    xt, wqt, wkt, wvt, wot, maskt, outp = io
    xt_r = xt.ap()
    Exp = mybir.ActivationFunctionType.Exp
    r_ = f"r{rep}_"

    with tc.tile_pool(name=r_ + "persist", bufs=1) as persist, \
         tc.tile_pool(name=r_ + "pj_ps", bufs=1, space="PSUM") as pj_ps, \
         tc.tile_pool(name=r_ + "sc_ps", bufs=3, space="PSUM") as sc_ps, \
         tc.tile_pool(name=r_ + "out_ps", bufs=1, space="PSUM") as out_ps, \
         tc.tile_pool(name=r_ + "psf_ps", bufs=2, space="PSUM") as psf_ps, \
         tc.tile_pool(name=r_ + "xt_p", bufs=3) as xt_p, \
         tc.tile_pool(name=r_ + "exp_p", bufs=8) as exp_p, \
         tc.tile_pool(name=r_ + "sums_p", bufs=3) as sums_p, \
         tc.tile_pool(name=r_ + "stg_p", bufs=2) as stg_p:

        qT = persist.tile([128, SF], BF16)      # [pair-dim d, s]
        kT = persist.tile([128, SF], BF16)
        vN = persist.tile([128, 32, 130], BF16)  # [s%128, s-tile, vA|1|vB|1]
        oT = persist.tile([128, SF], BF16)      # normalized attn out, T
        wq_s = persist.tile([128, 8, 128], BF16)
        wk_s = persist.tile([128, 8, 128], BF16)
        wv_s = persist.tile([128, 8, 128], BF16)
        wo_s = persist.tile([128, D], BF16)
        mk_s = persist.tile([128, 128], BF16)   # multiplicative 0/1 causal

        # DMA queue discipline: loads are always-ready at issue (WAR deps long
        # satisfied) and go on the ACT queue; stores wait on staging copies
        # and go on the SP queue, which dispatches nothing else — a waiting
        # DMA holds its queue's sequencer, so stores must not share a queue
        # with loads or compute dispatch.
        wq_r = wqt.ap().rearrange("(t p) m -> p t m", p=128)
        nc.sync.dma_start(wq_s[:, 0:1, :], wq_r[:, 0:1, :])
        # prefetch first x chunk in parallel on the other queue so the
        # first matmul can start early
        xti0 = xt_p.tile([128, 8, CH], BF16, name=f"xti_{rep}_0", tag="xti")
        nc.scalar.dma_start(xti0[:, 0:1, :], xt_r[0, 0:1].rearrange("t p s -> p t s"))
        nc.sync.dma_start(wq_s[:, 1:8, :], wq_r[:, 1:8, :])
        nc.scalar.dma_start(xti0[:, 1:3, :], xt_r[0, 1:3].rearrange("t p s -> p t s"))
        nc.scalar.dma_start(xti0[:, 3:8, :], xt_r[0, 3:8].rearrange("t p s -> p t s"))
        nc.sync.dma_start(wk_s[:], wkt.ap().rearrange("(t p) m -> p t m", p=128))
        nc.sync.dma_start(wv_s[:], wvt.ap().rearrange("(t p) m -> p t m", p=128))
        nc.sync.dma_start(wo_s[:], wot.ap())
        nc.sync.dma_start(mk_s[:], maskt.ap())
        ones32 = persist.tile([128, 32], BF16)
        nc.vector.memset(ones32[:], 1.0)
        nc.vector.tensor_copy(vN[:, :, 64:65], ones32[:].unsqueeze(2))
        nc.vector.tensor_copy(vN[:, :, 129:130], ones32[:].unsqueeze(2))

        def proj_chunk(sc):
            """Project s-chunk sc (512 rows of flat s) into qT/kT/vN."""
            if sc == 0:
                xti = xti0
            else:
                xti = xt_p.tile([128, 8, CH], BF16, name=f"xti_{rep}_{sc}",
                                tag="xti")
                nc.scalar.dma_start(xti[:], xt_r[sc].rearrange("t p s -> p t s"))
            col = slice(sc * CH, (sc + 1) * CH)

            psq = pj_ps.tile([128, CH], F32, tag="pj", name=f"psq_{rep}_{sc}")
            for t in range(8):
                nc.tensor.matmul(psq[:], wq_s[:, t, :], xti[:, t, :],
                                 start=(t == 0), stop=(t == 7))
            nc.vector.tensor_copy(qT[:, col], psq[:])

            psk = pj_ps.tile([128, CH], F32, tag="pj", name=f"psk_{rep}_{sc}")
            for t in range(8):
                nc.tensor.matmul(psk[:], wk_s[:, t, :], xti[:, t, :],
                                 start=(t == 0), stop=(t == 7))
            nc.vector.tensor_copy(kT[:, col], psk[:])

            # v, directly transposed: out[s%128, vdim] per 128-row k-tile
            psv = pj_ps.tile([128, CH], F32, tag="pj", name=f"psv_{rep}_{sc}")
            for j in range(4):
                for t in range(8):
                    nc.tensor.matmul(psv[:, j * 128:(j + 1) * 128],
                                     xti[:, t, j * 128:(j + 1) * 128],
                                     wv_s[:, t, :],
                                     start=(t == 0), stop=(t == 7))
            st = sc * 4
            dst = vN[:, st:st + 4, :].rearrange(
                "p t (h x) -> p t h x", h=2)[:, :, :, 0:64]
            src = psv[:].rearrange("p (t h x) -> p t h x", t=4, h=2)
            nc.vector.tensor_copy(dst, src)

        def attn_qchunk(b, qc, ci):
            """Attention + normalize + wo for q-chunk qc of batch b."""
            bcol = b * S
            qsl = slice(bcol + qc * CH, bcol + (qc + 1) * CH)
            nkt = 4 * (qc + 1)
            ps_o = [out_ps.tile([65, CH], F32, tag=f"ps_o{i}",
                                name=f"ps_o{i}_{rep}_{b}_{qc}")
                    for i in range(2)]
            for kt in range(nkt):
                # diag structure: r = offset of k-tile within the q-chunk
                r = kt * KT - qc * CH  # in {.., <0 full, 0,128,256,384 diag}
                r0 = max(r, 0)
                for hp in range(2):
                    hsl = slice(hp * 64, hp * 64 + 64)
                    ps_m = sc_ps.tile([128, CH], F32, tag="ps_s",
                                      name=f"ps_m_{rep}_{b}_{qc}_{kt}_{hp}")
                    et = exp_p.tile([128, CH], BF16, tag="et",
                                    name=f"et_{rep}_{b}_{qc}_{kt}_{hp}")
                    nc.tensor.matmul(
                        ps_m[:, r0:CH],
                        kT[hsl, bcol + kt * KT: bcol + (kt + 1) * KT],
                        qT[hsl, bcol + qc * CH + r0: bcol + (qc + 1) * CH],
                        start=True, stop=True)
                    nc.scalar.activation(et[:, r0:CH], ps_m[:, r0:CH],
                                         Exp, scale=0.125)
                    if r >= 0:
                        # multiplicative triangular mask on the diagonal 128
                        # columns, applied post-exp on bf16 SBUF (DVE fast
                        # mode; exp of unmasked scores is small so it's exact)
                        with tc.high_priority():
                            nc.vector.tensor_mul(et[:, r:r + 128],
                                                 et[:, r:r + 128],
                                                 mk_s[:, 0:128])
                    nc.tensor.matmul(
                        ps_o[hp][:, r0:CH],
                        vN[:, b * 16 + kt, hp * 65: hp * 65 + 65],
                        et[:, r0:CH],
                        start=(kt == 0), stop=(kt == nkt - 1),
                        skip_group_check=True)
            # copy unnormalized sums out of PSUM ASAP (frees the ps_o banks
            # for the next chunk's attn@v), then normalize from SBUF
            ob = [sums_p.tile([65, CH], F32, tag=f"ob{i}",
                              name=f"ob{i}_{rep}_{b}_{qc}")
                  for i in range(2)]
            with tc.high_priority():
                nc.vector.tensor_copy(ob[0][:], ps_o[0][:])
                nc.scalar.copy(ob[1][:], ps_o[1][:])
            with tc.high_priority():
                for hp in range(2):
                    rrow = sums_p.tile([1, CH], F32, tag="rrow",
                                       name=f"rrow_{rep}_{b}_{qc}_{hp}")
                    nc.vector.reciprocal(rrow[:], ob[hp][64:65, :])
                    bc = sums_p.tile([64, CH], F32, tag="bc",
                                     name=f"bc_{rep}_{b}_{qc}_{hp}")
                    nc.gpsimd.partition_broadcast(bc[:], rrow[0:1, :])
                    nc.gpsimd.tensor_mul(
                        oT[hp * 64: hp * 64 + 64, qsl],
                        ob[hp][0:64, :], bc[:])

        def wo_phase(b, qc, ci):
            """Final projection partial for chunk (b, qc): two 128-row blocks
            per store. Emitted one chunk late so its PE work fills the next
            chunk's attention gaps."""
            bcol = b * S
            for sp in range(2):
                stg = stg_p.tile([128, 2, D], BF16, tag="stg",
                                 name=f"stg_{rep}_{b}_{qc}_{sp}")
                for st4 in range(2):
                    soff = bcol + qc * CH + (sp * 2 + st4) * 128
                    for chn in range(2):
                        psf = psf_ps.tile([128, CH], F32, tag="psf",
                                          name=f"psf_{rep}_{b}_{qc}_{sp}_{st4}_{chn}")
                        nc.tensor.matmul(psf[:],
                                         oT[:, soff: soff + 128],
                                         wo_s[:, chn * CH:(chn + 1) * CH],
                                         start=True, stop=True)
                        dst = stg[:, st4, chn * CH:(chn + 1) * CH]
                        nc.vector.tensor_copy(dst, psf[:])
                soff0 = bcol + qc * CH + sp * 256
                nc.sync.dma_start(
                    outp.ap()[soff0: soff0 + 256, :].rearrange(
                        "(t p) m -> p t m", p=128),
                    stg[:])

        # interleaved pipeline: batches alternate, big q-chunks in the middle
        # (so proj + deferred-wo filler covers their attention gaps), small
        # qc=1 chunks last. proj s-chunk JIT two attention slots ahead; each
        # chunk's wo phase is emitted during the NEXT chunk's attention.
        attn_order = [(0, 0), (1, 0), (0, 2), (1, 2), (0, 3), (1, 3),
                      (0, 1), (1, 1)]
        proj_order = [0, 4, 1, 2, 5, 6, 3, 7]
        proj_chunk(proj_order[0])
        proj_chunk(proj_order[1])
        for i, (b, qc) in enumerate(attn_order):
            if i + 2 < len(proj_order):
                proj_chunk(proj_order[i + 2])
            attn_qchunk(b, qc, i)
            if i > 1:
                wo_phase(*attn_order[i - 2], i - 2)
        wo_phase(*attn_order[-2], len(attn_order) - 2)
        wo_phase(*attn_order[-1], len(attn_order) - 1)


def _build(repeats=1):
    nc = bacc.Bacc("TRN2", target_bir_lowering=False, debug=False)
    xt = nc.dram_tensor("xt", [SF // CH, 8, 128, CH], BF16,
                        kind="ExternalInput")
    wqt = nc.dram_tensor("wqt", [D, 128], BF16, kind="ExternalInput")
    wkt = nc.dram_tensor("wkt", [D, 128], BF16, kind="ExternalInput")
    wvt = nc.dram_tensor("wvt", [D, 128], BF16, kind="ExternalInput")
    wot = nc.dram_tensor("wot", [128, D], BF16, kind="ExternalInput")
    maskt = nc.dram_tensor("maskt", [128, 128], BF16, kind="ExternalInput")
    outp = nc.dram_tensor("outp", [SF, D], BF16, kind="ExternalOutput")
    io = (xt, wqt, wkt, wvt, wot, maskt, outp)

    with tile.TileContext(nc) as tc:
        for rep in range(repeats):
            _emit_body(nc, tc, io, rep)
    nc.compile()
    return nc


def _causal_mask_tile() -> np.ndarray:
    # multiplicative 0/1 mask: keep kp <= c within the diagonal block
    kp = np.arange(128)[:, None]
    c = np.arange(128)[None, :]
    return (kp <= c).astype(ml_dtypes.bfloat16)


def make_in_maps(x, wq, wk, wv, wo):
    bf = ml_dtypes.bfloat16
    # xt_arr[sc, t, p, s] = x[sc*CH + s, t*128 + p] — each sc block is a
    # contiguous 1MB DMA source
    xt = np.ascontiguousarray(
        x.reshape(SF // CH, CH, 8, 128).transpose(0, 2, 3, 1)).astype(bf)
    mask = _causal_mask_tile()
    in_maps = []
    for c in range(NCORES):
        rows = slice(c * 128, (c + 1) * 128)
        in_maps.append({
            "xt": xt,
            "wqt": np.ascontiguousarray(wq[rows, :].T).astype(bf),
            "wkt": np.ascontiguousarray(wk[rows, :].T).astype(bf),
            "wvt": np.ascontiguousarray(wv[rows, :].T).astype(bf),
            "wot": np.ascontiguousarray(wo[:, rows].T).astype(bf),
            "maskt": mask,
        })
    return in_maps


def _make_runner(nc):
    """Build a cached jitted PJRT runner. xt/maskt are replicated (same data
    on every core); weight slices are sharded per core; outputs unsharded on
    host. No donation: the zero output-init buffers stay resident on device
    across calls (the kernel writes every output element)."""
    import jax
    from jax.sharding import Mesh, PartitionSpec, NamedSharding
    try:
        from jax.experimental.shard_map import shard_map
    except ImportError:
        shard_map = jax.shard_map
    from concourse.bass2jax import (_bass_exec_p, install_neuronx_cc_hook,
                                    partition_id_tensor)

    install_neuronx_cc_hook()
    pname = nc.partition_id_tensor.name if nc.partition_id_tensor else None
    in_names, out_names, out_avals, zero_shapes = [], [], [], []
    for alloc in nc.m.functions[0].allocations:
        if not isinstance(alloc, mybir.MemoryLocationSet):
            continue
        name = alloc.memorylocations[0].name
        if alloc.kind == "ExternalInput":
            if name != pname:
                in_names.append(name)
        elif alloc.kind == "ExternalOutput":
            out_names.append(name)
            shape = tuple(alloc.tensor_shape)
            dtype = mybir.dt.np(alloc.dtype)
            out_avals.append(jax.core.ShapedArray(shape, dtype))
            zero_shapes.append((shape, dtype))
    n_params = len(in_names)
    all_in_names = in_names + out_names
    if pname is not None:
        all_in_names = all_in_names + [pname]

    def _body(*args):
        operands = list(args)
        if pname is not None:
            operands.append(partition_id_tensor())
        return tuple(_bass_exec_p.bind(
            *operands,
            out_avals=tuple(out_avals),
            in_names=tuple(all_in_names),
            out_names=tuple(out_names),
            lowering_input_output_aliases=(),
            sim_require_finite=True,
            sim_require_nnan=True,
            nc=nc,
        ))

    devices = jax.devices()[:NCORES]
    mesh = Mesh(np.asarray(devices), ("core",))
    shard = PartitionSpec("core")
    repl = PartitionSpec()
    REPLICATED = ("xt", "maskt")
    in_specs = tuple(repl if n in REPLICATED else shard for n in in_names) \
        + (shard,) * len(out_names)
    sharded = jax.jit(
        shard_map(_body, mesh=mesh, in_specs=in_specs,
                  out_specs=(shard,) * len(out_names), check_rep=False),
        keep_unused=True)
    zeros = [jax.device_put(np.zeros((NCORES * s[0], *s[1:]), d),
                            NamedSharding(mesh, shard))
             for (s, d) in zero_shapes]
    jax.block_until_ready(zeros)

    def run(in_maps):
        args = []
        for n in in_names:
            if n in REPLICATED:
                args.append(jax.device_put(np.asarray(in_maps[0][n]),
                                           NamedSharding(mesh, repl)))
            else:
                args.append(jax.device_put(
                    np.concatenate([np.asarray(m[n]) for m in in_maps], axis=0),
                    NamedSharding(mesh, shard)))
        outs = sharded(*args, *zeros)
        return [
            {n: np.asarray(outs[i]).reshape(NCORES, *out_avals[i].shape)[c]
             for i, n in enumerate(out_names)}
            for c in range(NCORES)
        ]

    return run


def kernel(x, wq, wk, wv, wo):
    x = np.asarray(x, dtype=np.float32)
    wq = np.asarray(wq, dtype=np.float32)
    wk = np.asarray(wk, dtype=np.float32)
    wv = np.asarray(wv, dtype=np.float32)
    wo = np.asarray(wo, dtype=np.float32)

    if "nc" not in _cache:
        _cache["nc"] = _build()
    nc = _cache["nc"]
    in_maps = make_in_maps(x, wq, wk, wv, wo)

    try:
        if "run" not in _cache:
            _cache["run"] = _make_runner(nc)
        results = _cache["run"](in_maps)
    except Exception:
        _cache.pop("run", None)
        results = run_bass_kernel_spmd(
            nc, in_maps, core_ids=list(range(NCORES))).results

    out = np.zeros((SF, D), dtype=np.float64)
    for r in results:
        out += r["outp"].astype(np.float64)
    return out.astype(np.float32).reshape(B, S, D)


# revision 28
# speedup vs baseline: 1.9587x; 1.9587x over previous
"""Causal multi-head attention (B=2, S=2048, D=1024, H=16) on 8 NeuronCores.

Sharding: head-parallel. Core c owns heads {2c, 2c+1} = a 128-wide slice of
the q/k/v projection output dims and of wo's input dim. Each core computes
attention for its 2 heads over both batch elements and a full-size partial
of the final projection; the host sums the 8 partials.

v2 (all-bf16): inputs, weights, internal operand tiles, and the output
partials are bf16 (fp32 PSUM accumulation everywhere), halving HBM traffic
and enabling full-rate PE on every tile size. V is projected directly in
transposed layout (stationary = x-block, moving = wv), eliminating the PE
transpose pass. Scores are computed transposed (scoresT[k, q]) so softmax
probs feed attn@v without transposition; a ones-column in vN makes the same
matmul emit softmax denominators. |scores/8| < ~3 so exp without
max-subtraction is exact. DMAs are batched (1 load/chunk, 2-block stores)
and issued from both SP and ACT queues to avoid sequencer serialization.
Engine balance: PE does matmuls only; ACT does exp + half the output-stage
copies; DVE does qk/v copies, reciprocal + normalize; Pool (gpsimd) does
diag masks, denominator broadcast, and the other output-stage copies.
"""
import numpy as np
import ml_dtypes

import concourse.bass as bass
import concourse.tile as tile
from concourse import bacc, mybir
from concourse.bass_utils import run_bass_kernel_spmd

B, S, D = 2, 2048, 1024
H, HD = 16, 64
NCORES = 8
SF = B * S              # 4096 flattened rows
CH = 512                # column chunk for matmuls
KT = 128                # k-tile (keys per tile)
NEG = -1.0e38

F32 = mybir.dt.float32
BF16 = mybir.dt.bfloat16

_cache = {}


def _emit_body(nc, tc, io, rep):
    xt, wqt, wkt, wvt, wot, maskt, outp = io
    xt_r = xt.ap()
    Exp = mybir.ActivationFunctionType.Exp
    r_ = f"r{rep}_"

    with tc.tile_pool(name=r_ + "persist", bufs=1) as persist, \
         tc.tile_pool(name=r_ + "pj_ps", bufs=1, space="PSUM") as pj_ps, \
         tc.tile_pool(name=r_ + "sc_ps", bufs=3, space="PSUM") as sc_ps, \
         tc.tile_pool(name=r_ + "out_ps", bufs=1, space="PSUM") as out_ps, \
         tc.tile_pool(name=r_ + "psf_ps", bufs=2, space="PSUM") as psf_ps, \
         tc.tile_pool(name=r_ + "xt_p", bufs=3) as xt_p, \
         tc.tile_pool(name=r_ + "exp_p", bufs=8) as exp_p, \
         tc.tile_pool(name=r_ + "sums_p", bufs=3) as sums_p, \
         tc.tile_pool(name=r_ + "stg_p", bufs=2) as stg_p:

        qT = persist.tile([128, SF], BF16)      # [pair-dim d, s]
        kT = persist.tile([128, SF], BF16)
        vN = persist.tile([128, 32, 130], BF16)  # [s%128, s-tile, vA|1|vB|1]
        oT = persist.tile([128, SF], BF16)      # normalized attn out, T
        wq_s = persist.tile([128, 8, 128], BF16)
        wk_s = persist.tile([128, 8, 128], BF16)
        wv_s = persist.tile([128, 8, 128], BF16)
        wo_s = persist.tile([128, D], BF16)
        mk_s = persist.tile([128, 128], BF16)   # multiplicative 0/1 causal

        # DMA queue discipline: loads are always-ready at issue (WAR deps long
        # satisfied) and go on the ACT queue; stores wait on staging copies
        # and go on the SP queue, which dispatches nothing else — a waiting
        # DMA holds its queue's sequencer, so stores must not share a queue
        # with loads or compute dispatch.
        wq_r = wqt.ap().rearrange("(t p) m -> p t m", p=128)
        nc.sync.dma_start(wq_s[:, 0:1, :], wq_r[:, 0:1, :])
        # prefetch first x chunk in parallel on the other queue so the
        # first matmul can start early
        xti0 = xt_p.tile([128, 8, CH], BF16, name=f"xti_{rep}_0", tag="xti")
        nc.scalar.dma_start(xti0[:, 0:1, :], xt_r[0, 0:1].rearrange("t p s -> p t s"))
        nc.sync.dma_start(wq_s[:, 1:8, :], wq_r[:, 1:8, :])
        nc.scalar.dma_start(xti0[:, 1:3, :], xt_r[0, 1:3].rearrange("t p s -> p t s"))
        nc.scalar.dma_start(xti0[:, 3:8, :], xt_r[0, 3:8].rearrange("t p s -> p t s"))
        nc.sync.dma_start(wk_s[:], wkt.ap().rearrange("(t p) m -> p t m", p=128))
        nc.sync.dma_start(wv_s[:], wvt.ap().rearrange("(t p) m -> p t m", p=128))
        nc.sync.dma_start(wo_s[:], wot.ap())
        nc.sync.dma_start(mk_s[:], maskt.ap())
        ones32 = persist.tile([128, 32], BF16)
        nc.vector.memset(ones32[:], 1.0)
        nc.vector.tensor_copy(vN[:, :, 64:65], ones32[:].unsqueeze(2))
        nc.vector.tensor_copy(vN[:, :, 129:130], ones32[:].unsqueeze(2))

        def proj_chunk(sc):
            """Project s-chunk sc (512 rows of flat s) into qT/kT/vN."""
            if sc == 0:
                xti = xti0
            else:
                xti = xt_p.tile([128, 8, CH], BF16, name=f"xti_{rep}_{sc}",
                                tag="xti")
                nc.scalar.dma_start(xti[:], xt_r[sc].rearrange("t p s -> p t s"))
            col = slice(sc * CH, (sc + 1) * CH)

            psq = pj_ps.tile([128, CH], F32, tag="pj", name=f"psq_{rep}_{sc}")
            for t in range(8):
                nc.tensor.matmul(psq[:], wq_s[:, t, :], xti[:, t, :],
                                 start=(t == 0), stop=(t == 7))
            nc.vector.tensor_copy(qT[:, col], psq[:])

            psk = pj_ps.tile([128, CH], F32, tag="pj", name=f"psk_{rep}_{sc}")
            for t in range(8):
                nc.tensor.matmul(psk[:], wk_s[:, t, :], xti[:, t, :],
                                 start=(t == 0), stop=(t == 7))
            nc.vector.tensor_copy(kT[:, col], psk[:])

            # v, directly transposed: out[s%128, vdim] per 128-row k-tile
            psv = pj_ps.tile([128, CH], F32, tag="pj", name=f"psv_{rep}_{sc}")
            for j in range(4):
                for t in range(8):
                    nc.tensor.matmul(psv[:, j * 128:(j + 1) * 128],
                                     xti[:, t, j * 128:(j + 1) * 128],
                                     wv_s[:, t, :],
                                     start=(t == 0), stop=(t == 7))
            st = sc * 4
            dst = vN[:, st:st + 4, :].rearrange(
                "p t (h x) -> p t h x", h=2)[:, :, :, 0:64]
            src = psv[:].rearrange("p (t h x) -> p t h x", t=4, h=2)
            nc.vector.tensor_copy(dst, src)

        def attn_qchunk(b, qc, ci):
            """Attention + normalize + wo for q-chunk qc of batch b."""
            bcol = b * S
            qsl = slice(bcol + qc * CH, bcol + (qc + 1) * CH)
            nkt = 4 * (qc + 1)
            ps_o = [out_ps.tile([65, CH], F32, tag=f"ps_o{i}",
                                name=f"ps_o{i}_{rep}_{b}_{qc}")
                    for i in range(2)]
            for kt in range(nkt):
                # diag structure: r = offset of k-tile within the q-chunk
                r = kt * KT - qc * CH  # in {.., <0 full, 0,128,256,384 diag}
                r0 = max(r, 0)
                for hp in range(2):
                    hsl = slice(hp * 64, hp * 64 + 64)
                    ps_m = sc_ps.tile([128, CH], F32, tag="ps_s",
                                      name=f"ps_m_{rep}_{b}_{qc}_{kt}_{hp}")
                    et = exp_p.tile([128, CH], BF16, tag="et",
                                    name=f"et_{rep}_{b}_{qc}_{kt}_{hp}")
                    nc.tensor.matmul(
                        ps_m[:, r0:CH],
                        kT[hsl, bcol + kt * KT: bcol + (kt + 1) * KT],
                        qT[hsl, bcol + qc * CH + r0: bcol + (qc + 1) * CH],
                        start=True, stop=True)
                    nc.scalar.activation(et[:, r0:CH], ps_m[:, r0:CH],
                                         Exp, scale=0.125)
                    if r >= 0:
                        # multiplicative triangular mask on the diagonal 128
                        # columns, applied post-exp on bf16 SBUF (DVE fast
                        # mode; exp of unmasked scores is small so it's exact)
                        with tc.high_priority():
                            nc.vector.tensor_mul(et[:, r:r + 128],
                                                 et[:, r:r + 128],
                                                 mk_s[:, 0:128])
                    nc.tensor.matmul(
                        ps_o[hp][:, r0:CH],
                        vN[:, b * 16 + kt, hp * 65: hp * 65 + 65],
                        et[:, r0:CH],
                        start=(kt == 0), stop=(kt == nkt - 1),
                        skip_group_check=True)
            # copy unnormalized sums out of PSUM ASAP (frees the ps_o banks
            # for the next chunk's attn@v), then normalize from SBUF
            ob = [sums_p.tile([65, CH], F32, tag=f"ob{i}",
                              name=f"ob{i}_{rep}_{b}_{qc}")
                  for i in range(2)]
            with tc.high_priority():
                nc.vector.tensor_copy(ob[0][:], ps_o[0][:])
                nc.scalar.copy(ob[1][:], ps_o[1][:])
            with tc.high_priority():
                for hp in range(2):
                    rrow = sums_p.tile([1, CH], F32, tag="rrow",
                                       name=f"rrow_{rep}_{b}_{qc}_{hp}")
                    nc.vector.reciprocal(rrow[:], ob[hp][64:65, :])
                    bc = sums_p.tile([64, CH], F32, tag="bc",
                                     name=f"bc_{rep}_{b}_{qc}_{hp}")
                    nc.gpsimd.partition_broadcast(bc[:], rrow[0:1, :])
                    nc.vector.tensor_mul(
                        oT[hp * 64: hp * 64 + 64, qsl],
                        ob[hp][0:64, :], bc[:])

        def wo_phase(b, qc, ci):
            """Final projection partial for chunk (b, qc): two 128-row blocks
            per store. Emitted one chunk late so its PE work fills the next
            chunk's attention gaps."""
            bcol = b * S
            for sp in range(2):
                stg = stg_p.tile([128, 2, D], BF16, tag="stg",
                                 name=f"stg_{rep}_{b}_{qc}_{sp}")
                for st4 in range(2):
                    soff = bcol + qc * CH + (sp * 2 + st4) * 128
                    for chn in range(2):
                        psf = psf_ps.tile([128, CH], F32, tag="psf",
                                          name=f"psf_{rep}_{b}_{qc}_{sp}_{st4}_{chn}")
                        nc.tensor.matmul(psf[:],
                                         oT[:, soff: soff + 128],
                                         wo_s[:, chn * CH:(chn + 1) * CH],
                                         start=True, stop=True)
                        dst = stg[:, st4, chn * CH:(chn + 1) * CH]
                        nc.vector.tensor_copy(dst, psf[:])
                soff0 = bcol + qc * CH + sp * 256
                nc.sync.dma_start(
                    outp.ap()[soff0: soff0 + 256, :].rearrange(
                        "(t p) m -> p t m", p=128),
                    stg[:])

        # interleaved pipeline: batches alternate, big q-chunks in the middle
        # (so proj + deferred-wo filler covers their attention gaps), small
        # qc=1 chunks last. proj s-chunk JIT two attention slots ahead; each
        # chunk's wo phase is emitted during the NEXT chunk's attention.
        attn_order = [(0, 0), (1, 0), (0, 2), (1, 2), (0, 3), (1, 3),
                      (0, 1), (1, 1)]
        proj_order = [0, 4, 1, 2, 5, 6, 3, 7]
        proj_chunk(proj_order[0])
        proj_chunk(proj_order[1])
        for i, (b, qc) in enumerate(attn_order):
            if i + 2 < len(proj_order):
                proj_chunk(proj_order[i + 2])
            attn_qchunk(b, qc, i)
            if i > 1:
                wo_phase(*attn_order[i - 2], i - 2)
        wo_phase(*attn_order[-2], len(attn_order) - 2)
        wo_phase(*attn_order[-1], len(attn_order) - 1)


def _build(repeats=1):
    nc = bacc.Bacc("TRN2", target_bir_lowering=False, debug=False)
    xt = nc.dram_tensor("xt", [SF // CH, 8, 128, CH], BF16,
                        kind="ExternalInput")
    wqt = nc.dram_tensor("wqt", [D, 128], BF16, kind="ExternalInput")
    wkt = nc.dram_tensor("wkt", [D, 128], BF16, kind="ExternalInput")
    wvt = nc.dram_tensor("wvt", [D, 128], BF16, kind="ExternalInput")
    wot = nc.dram_tensor("wot", [128, D], BF16, kind="ExternalInput")
    maskt = nc.dram_tensor("maskt", [128, 128], BF16, kind="ExternalInput")
    outp = nc.dram_tensor("outp", [SF, D], BF16, kind="ExternalOutput")
    io = (xt, wqt, wkt, wvt, wot, maskt, outp)

    with tile.TileContext(nc) as tc:
        for rep in range(repeats):
            _emit_body(nc, tc, io, rep)
    nc.compile()
    return nc


def _causal_mask_tile() -> np.ndarray:
    # multiplicative 0/1 mask: keep kp <= c within the diagonal block
    kp = np.arange(128)[:, None]
    c = np.arange(128)[None, :]
    return (kp <= c).astype(ml_dtypes.bfloat16)


def make_in_maps(x, wq, wk, wv, wo):
    bf = ml_dtypes.bfloat16
    # xt_arr[sc, t, p, s] = x[sc*CH + s, t*128 + p] — each sc block is a
    # contiguous 1MB DMA source
    xt = np.ascontiguousarray(
        x.reshape(SF // CH, CH, 8, 128).transpose(0, 2, 3, 1)).astype(bf)
    mask = _causal_mask_tile()
    in_maps = []
    for c in range(NCORES):
        rows = slice(c * 128, (c + 1) * 128)
        in_maps.append({
            "xt": xt,
            "wqt": np.ascontiguousarray(wq[rows, :].T).astype(bf),
            "wkt": np.ascontiguousarray(wk[rows, :].T).astype(bf),
            "wvt": np.ascontiguousarray(wv[rows, :].T).astype(bf),
            "wot": np.ascontiguousarray(wo[:, rows].T).astype(bf),
            "maskt": mask,
        })
    return in_maps


def _make_runner(nc):
    """Build a cached jitted PJRT runner. xt/maskt are replicated (same data
    on every core); weight slices are sharded per core; outputs unsharded on
    host. No donation: the zero output-init buffers stay resident on device
    across calls (the kernel writes every output element)."""
    import jax
    from jax.sharding import Mesh, PartitionSpec, NamedSharding
    try:
        from jax.experimental.shard_map import shard_map
    except ImportError:
        shard_map = jax.shard_map
    from concourse.bass2jax import (_bass_exec_p, install_neuronx_cc_hook,
                                    partition_id_tensor)

    install_neuronx_cc_hook()
    pname = nc.partition_id_tensor.name if nc.partition_id_tensor else None
    in_names, out_names, out_avals, zero_shapes = [], [], [], []
    for alloc in nc.m.functions[0].allocations:
        if not isinstance(alloc, mybir.MemoryLocationSet):
            continue
        name = alloc.memorylocations[0].name
        if alloc.kind == "ExternalInput":
            if name != pname:
                in_names.append(name)
        elif alloc.kind == "ExternalOutput":
            out_names.append(name)
            shape = tuple(alloc.tensor_shape)
            dtype = mybir.dt.np(alloc.dtype)
            out_avals.append(jax.core.ShapedArray(shape, dtype))
            zero_shapes.append((shape, dtype))
    n_params = len(in_names)
    all_in_names = in_names + out_names
    if pname is not None:
        all_in_names = all_in_names + [pname]

    def _body(*args):
        operands = list(args)
        if pname is not None:
            operands.append(partition_id_tensor())
        return tuple(_bass_exec_p.bind(
            *operands,
            out_avals=tuple(out_avals),
            in_names=tuple(all_in_names),
            out_names=tuple(out_names),
            lowering_input_output_aliases=(),
            sim_require_finite=True,
            sim_require_nnan=True,
            nc=nc,
        ))

    devices = jax.devices()[:NCORES]
    mesh = Mesh(np.asarray(devices), ("core",))
    shard = PartitionSpec("core")
    repl = PartitionSpec()
    REPLICATED = ("xt", "maskt")
    in_specs = tuple(repl if n in REPLICATED else shard for n in in_names) \
        + (shard,) * len(out_names)
    sharded = jax.jit(
        shard_map(_body, mesh=mesh, in_specs=in_specs,
                  out_specs=(shard,) * len(out_names), check_rep=False),
        keep_unused=True)
    zeros = [jax.device_put(np.zeros((NCORES * s[0], *s[1:]), d),
                            NamedSharding(mesh, shard))
             for (s, d) in zero_shapes]
    jax.block_until_ready(zeros)

    def run(in_maps):
        args = []
        for n in in_names:
            if n in REPLICATED:
                args.append(jax.device_put(np.asarray(in_maps[0][n]),
                                           NamedSharding(mesh, repl)))
            else:
                args.append(jax.device_put(
                    np.concatenate([np.asarray(m[n]) for m in in_maps], axis=0),
                    NamedSharding(mesh, shard)))
        outs = sharded(*args, *zeros)
        return [
            {n: np.asarray(outs[i]).reshape(NCORES, *out_avals[i].shape)[c]
             for i, n in enumerate(out_names)}
            for c in range(NCORES)
        ]

    return run


def kernel(x, wq, wk, wv, wo):
    x = np.asarray(x, dtype=np.float32)
    wq = np.asarray(wq, dtype=np.float32)
    wk = np.asarray(wk, dtype=np.float32)
    wv = np.asarray(wv, dtype=np.float32)
    wo = np.asarray(wo, dtype=np.float32)

    if "nc" not in _cache:
        _cache["nc"] = _build()
    nc = _cache["nc"]
    in_maps = make_in_maps(x, wq, wk, wv, wo)

    try:
        if "run" not in _cache:
            _cache["run"] = _make_runner(nc)
        results = _cache["run"](in_maps)
    except Exception:
        _cache.pop("run", None)
        results = run_bass_kernel_spmd(
            nc, in_maps, core_ids=list(range(NCORES))).results

    out = np.zeros((SF, D), dtype=np.float64)
    for r in results:
        out += r["outp"].astype(np.float64)
    return out.astype(np.float32).reshape(B, S, D)


# revision 29
# speedup vs baseline: 2.0641x; 1.0538x over previous
"""Causal multi-head attention (B=2, S=2048, D=1024, H=16) on 8 NeuronCores.

Sharding: head-parallel. Core c owns heads {2c, 2c+1} = a 128-wide slice of
the q/k/v projection output dims and of wo's input dim. Each core computes
attention for its 2 heads over both batch elements and a full-size partial
of the final projection; the host sums the 8 partials.

v2 (all-bf16): inputs, weights, internal operand tiles, and the output
partials are bf16 (fp32 PSUM accumulation everywhere), halving HBM traffic
and enabling full-rate PE on every tile size. V is projected directly in
transposed layout (stationary = x-block, moving = wv), eliminating the PE
transpose pass. Scores are computed transposed (scoresT[k, q]) so softmax
probs feed attn@v without transposition; a ones-column in vN makes the same
matmul emit softmax denominators. |scores/8| < ~3 so exp without
max-subtraction is exact. DMAs are batched (1 load/chunk, 2-block stores)
and issued from both SP and ACT queues to avoid sequencer serialization.
Engine balance: PE does matmuls only; ACT does exp + half the output-stage
copies; DVE does qk/v copies, reciprocal + normalize; Pool (gpsimd) does
diag masks, denominator broadcast, and the other output-stage copies.
"""
import numpy as np
import ml_dtypes

import concourse.bass as bass
import concourse.tile as tile
from concourse import bacc, mybir
from concourse.bass_utils import run_bass_kernel_spmd

B, S, D = 2, 2048, 1024
H, HD = 16, 64
NCORES = 8
SF = B * S              # 4096 flattened rows
CH = 512                # column chunk for matmuls
KT = 128                # k-tile (keys per tile)
NEG = -1.0e38

F32 = mybir.dt.float32
BF16 = mybir.dt.bfloat16

_cache = {}


def _emit_body(nc, tc, io, rep):
    xt, wqt, wkt, wvt, wot, maskt, outp = io
    xt_r = xt.ap()
    Exp = mybir.ActivationFunctionType.Exp
    r_ = f"r{rep}_"

    with tc.tile_pool(name=r_ + "persist", bufs=1) as persist, \
         tc.tile_pool(name=r_ + "pj_ps", bufs=1, space="PSUM") as pj_ps, \
         tc.tile_pool(name=r_ + "sc_ps", bufs=3, space="PSUM") as sc_ps, \
         tc.tile_pool(name=r_ + "out_ps", bufs=1, space="PSUM") as out_ps, \
         tc.tile_pool(name=r_ + "psf_ps", bufs=2, space="PSUM") as psf_ps, \
         tc.tile_pool(name=r_ + "xt_p", bufs=3) as xt_p, \
         tc.tile_pool(name=r_ + "exp_p", bufs=8) as exp_p, \
         tc.tile_pool(name=r_ + "sums_p", bufs=3) as sums_p, \
         tc.tile_pool(name=r_ + "stg_p", bufs=2) as stg_p:

        qT = persist.tile([128, SF], BF16)      # [pair-dim d, s]
        kT = persist.tile([128, SF], BF16)
        vN = persist.tile([128, 32, 130], BF16)  # [s%128, s-tile, vA|1|vB|1]
        oT = persist.tile([128, SF], BF16)      # normalized attn out, T
        wq_s = persist.tile([128, 8, 128], BF16)
        wk_s = persist.tile([128, 8, 128], BF16)
        wv_s = persist.tile([128, 8, 128], BF16)
        wo_s = persist.tile([128, D], BF16)
        mk_s = persist.tile([128, 128], BF16)   # multiplicative 0/1 causal

        # DMA queue discipline: loads are always-ready at issue (WAR deps long
        # satisfied) and go on the ACT queue; stores wait on staging copies
        # and go on the SP queue, which dispatches nothing else — a waiting
        # DMA holds its queue's sequencer, so stores must not share a queue
        # with loads or compute dispatch.
        wq_r = wqt.ap().rearrange("(t p) m -> p t m", p=128)
        nc.sync.dma_start(wq_s[:, 0:1, :], wq_r[:, 0:1, :])
        # prefetch first x chunk in parallel on the other queue so the
        # first matmul can start early
        xti0 = xt_p.tile([128, 8, CH], BF16, name=f"xti_{rep}_0", tag="xti")
        nc.scalar.dma_start(xti0[:, 0:1, :], xt_r[0, 0:1].rearrange("t p s -> p t s"))
        nc.sync.dma_start(wq_s[:, 1:8, :], wq_r[:, 1:8, :])
        nc.scalar.dma_start(xti0[:, 1:3, :], xt_r[0, 1:3].rearrange("t p s -> p t s"))
        nc.scalar.dma_start(xti0[:, 3:8, :], xt_r[0, 3:8].rearrange("t p s -> p t s"))
        nc.sync.dma_start(wk_s[:], wkt.ap().rearrange("(t p) m -> p t m", p=128))
        nc.sync.dma_start(wv_s[:], wvt.ap().rearrange("(t p) m -> p t m", p=128))
        nc.sync.dma_start(wo_s[:], wot.ap())
        nc.sync.dma_start(mk_s[:], maskt.ap())
        ones32 = persist.tile([128, 32], BF16)
        nc.vector.memset(ones32[:], 1.0)
        nc.vector.tensor_copy(vN[:, :, 64:65], ones32[:].unsqueeze(2))
        nc.vector.tensor_copy(vN[:, :, 129:130], ones32[:].unsqueeze(2))

        def proj_chunk(sc):
            """Project s-chunk sc (512 rows of flat s) into qT/kT/vN."""
            if sc == 0:
                xti = xti0
            else:
                xti = xt_p.tile([128, 8, CH], BF16, name=f"xti_{rep}_{sc}",
                                tag="xti")
                for tp2 in range(4):
                    t0, t1 = tp2 * 2, tp2 * 2 + 2
                    nc.gpsimd.dma_start(
                        xti[:, t0:t1, :],
                        xt_r[sc, t0:t1].rearrange("t p s -> p t s"))
            col = slice(sc * CH, (sc + 1) * CH)

            psq = pj_ps.tile([128, CH], F32, tag="pj", name=f"psq_{rep}_{sc}")
            for t in range(8):
                nc.tensor.matmul(psq[:], wq_s[:, t, :], xti[:, t, :],
                                 start=(t == 0), stop=(t == 7))
            nc.vector.tensor_copy(qT[:, col], psq[:])

            psk = pj_ps.tile([128, CH], F32, tag="pj", name=f"psk_{rep}_{sc}")
            for t in range(8):
                nc.tensor.matmul(psk[:], wk_s[:, t, :], xti[:, t, :],
                                 start=(t == 0), stop=(t == 7))
            nc.vector.tensor_copy(kT[:, col], psk[:])

            # v, directly transposed: out[s%128, vdim] per 128-row k-tile
            psv = pj_ps.tile([128, CH], F32, tag="pj", name=f"psv_{rep}_{sc}")
            for j in range(4):
                for t in range(8):
                    nc.tensor.matmul(psv[:, j * 128:(j + 1) * 128],
                                     xti[:, t, j * 128:(j + 1) * 128],
                                     wv_s[:, t, :],
                                     start=(t == 0), stop=(t == 7))
            st = sc * 4
            dst = vN[:, st:st + 4, :].rearrange(
                "p t (h x) -> p t h x", h=2)[:, :, :, 0:64]
            src = psv[:].rearrange("p (t h x) -> p t h x", t=4, h=2)
            nc.vector.tensor_copy(dst, src)

        def attn_qchunk(b, qc, ci):
            """Attention + normalize + wo for q-chunk qc of batch b."""
            bcol = b * S
            qsl = slice(bcol + qc * CH, bcol + (qc + 1) * CH)
            nkt = 4 * (qc + 1)
            ps_o = [out_ps.tile([65, CH], F32, tag=f"ps_o{i}",
                                name=f"ps_o{i}_{rep}_{b}_{qc}")
                    for i in range(2)]
            for kt in range(nkt):
                # diag structure: r = offset of k-tile within the q-chunk
                r = kt * KT - qc * CH  # in {.., <0 full, 0,128,256,384 diag}
                r0 = max(r, 0)
                for hp in range(2):
                    hsl = slice(hp * 64, hp * 64 + 64)
                    ps_m = sc_ps.tile([128, CH], F32, tag="ps_s",
                                      name=f"ps_m_{rep}_{b}_{qc}_{kt}_{hp}")
                    et = exp_p.tile([128, CH], BF16, tag="et",
                                    name=f"et_{rep}_{b}_{qc}_{kt}_{hp}")
                    nc.tensor.matmul(
                        ps_m[:, r0:CH],
                        kT[hsl, bcol + kt * KT: bcol + (kt + 1) * KT],
                        qT[hsl, bcol + qc * CH + r0: bcol + (qc + 1) * CH],
                        start=True, stop=True)
                    nc.scalar.activation(et[:, r0:CH], ps_m[:, r0:CH],
                                         Exp, scale=0.125)
                    if r >= 0:
                        # multiplicative triangular mask on the diagonal 128
                        # columns, applied post-exp on bf16 SBUF (DVE fast
                        # mode; exp of unmasked scores is small so it's exact)
                        with tc.high_priority():
                            nc.vector.tensor_mul(et[:, r:r + 128],
                                                 et[:, r:r + 128],
                                                 mk_s[:, 0:128])
                    nc.tensor.matmul(
                        ps_o[hp][:, r0:CH],
                        vN[:, b * 16 + kt, hp * 65: hp * 65 + 65],
                        et[:, r0:CH],
                        start=(kt == 0), stop=(kt == nkt - 1),
                        skip_group_check=True)
            # copy unnormalized sums out of PSUM ASAP (frees the ps_o banks
            # for the next chunk's attn@v), then normalize from SBUF
            ob = [sums_p.tile([65, CH], F32, tag=f"ob{i}",
                              name=f"ob{i}_{rep}_{b}_{qc}")
                  for i in range(2)]
            with tc.high_priority():
                nc.vector.tensor_copy(ob[0][:], ps_o[0][:])
                nc.scalar.copy(ob[1][:], ps_o[1][:])
            with tc.high_priority():
                for hp in range(2):
                    rrow = sums_p.tile([1, CH], F32, tag="rrow",
                                       name=f"rrow_{rep}_{b}_{qc}_{hp}")
                    nc.vector.reciprocal(rrow[:], ob[hp][64:65, :])
                    bc = sums_p.tile([64, CH], F32, tag="bc",
                                     name=f"bc_{rep}_{b}_{qc}_{hp}")
                    nc.gpsimd.partition_broadcast(bc[:], rrow[0:1, :])
                    nc.vector.tensor_mul(
                        oT[hp * 64: hp * 64 + 64, qsl],
                        ob[hp][0:64, :], bc[:])

        def wo_phase(b, qc, ci):
            """Final projection partial for chunk (b, qc): two 128-row blocks
            per store. Emitted one chunk late so its PE work fills the next
            chunk's attention gaps."""
            bcol = b * S
            for sp in range(2):
                stg = stg_p.tile([128, 2, D], BF16, tag="stg",
                                 name=f"stg_{rep}_{b}_{qc}_{sp}")
                for st4 in range(2):
                    soff = bcol + qc * CH + (sp * 2 + st4) * 128
                    for chn in range(2):
                        psf = psf_ps.tile([128, CH], F32, tag="psf",
                                          name=f"psf_{rep}_{b}_{qc}_{sp}_{st4}_{chn}")
                        nc.tensor.matmul(psf[:],
                                         oT[:, soff: soff + 128],
                                         wo_s[:, chn * CH:(chn + 1) * CH],
                                         start=True, stop=True)
                        dst = stg[:, st4, chn * CH:(chn + 1) * CH]
                        nc.vector.tensor_copy(dst, psf[:])
                soff0 = bcol + qc * CH + sp * 256
                nc.sync.dma_start(
                    outp.ap()[soff0: soff0 + 256, :].rearrange(
                        "(t p) m -> p t m", p=128),
                    stg[:])

        # interleaved pipeline: batches alternate, big q-chunks in the middle
        # (so proj + deferred-wo filler covers their attention gaps), small
        # qc=1 chunks last. proj s-chunk JIT two attention slots ahead; each
        # chunk's wo phase is emitted during the NEXT chunk's attention.
        attn_order = [(0, 0), (1, 0), (0, 2), (1, 2), (0, 3), (1, 3),
                      (0, 1), (1, 1)]
        proj_order = [0, 4, 1, 2, 5, 6, 3, 7]
        proj_chunk(proj_order[0])
        proj_chunk(proj_order[1])
        for i, (b, qc) in enumerate(attn_order):
            if i + 2 < len(proj_order):
                proj_chunk(proj_order[i + 2])
            attn_qchunk(b, qc, i)
            if i > 1:
                wo_phase(*attn_order[i - 2], i - 2)
        wo_phase(*attn_order[-2], len(attn_order) - 2)
        wo_phase(*attn_order[-1], len(attn_order) - 1)


def _build(repeats=1):
    nc = bacc.Bacc("TRN2", target_bir_lowering=False, debug=False)
    xt = nc.dram_tensor("xt", [SF // CH, 8, 128, CH], BF16,
                        kind="ExternalInput")
    wqt = nc.dram_tensor("wqt", [D, 128], BF16, kind="ExternalInput")
    wkt = nc.dram_tensor("wkt", [D, 128], BF16, kind="ExternalInput")
    wvt = nc.dram_tensor("wvt", [D, 128], BF16, kind="ExternalInput")
    wot = nc.dram_tensor("wot", [128, D], BF16, kind="ExternalInput")
    maskt = nc.dram_tensor("maskt", [128, 128], BF16, kind="ExternalInput")
    outp = nc.dram_tensor("outp", [SF, D], BF16, kind="ExternalOutput")
    io = (xt, wqt, wkt, wvt, wot, maskt, outp)

    with tile.TileContext(nc) as tc:
        for rep in range(repeats):
            _emit_body(nc, tc, io, rep)
    nc.compile()
    return nc


def _causal_mask_tile() -> np.ndarray:
    # multiplicative 0/1 mask: keep kp <= c within the diagonal block
    kp = np.arange(128)[:, None]
    c = np.arange(128)[None, :]
    return (kp <= c).astype(ml_dtypes.bfloat16)


def make_in_maps(x, wq, wk, wv, wo):
    bf = ml_dtypes.bfloat16
    # xt_arr[sc, t, p, s] = x[sc*CH + s, t*128 + p] — each sc block is a
    # contiguous 1MB DMA source
    xt = np.ascontiguousarray(
        x.reshape(SF // CH, CH, 8, 128).transpose(0, 2, 3, 1)).astype(bf)
    mask = _causal_mask_tile()
    in_maps = []
    for c in range(NCORES):
        rows = slice(c * 128, (c + 1) * 128)
        in_maps.append({
            "xt": xt,
            "wqt": np.ascontiguousarray(wq[rows, :].T).astype(bf),
            "wkt": np.ascontiguousarray(wk[rows, :].T).astype(bf),
            "wvt": np.ascontiguousarray(wv[rows, :].T).astype(bf),
            "wot": np.ascontiguousarray(wo[:, rows].T).astype(bf),
            "maskt": mask,
        })
    return in_maps


def _make_runner(nc):
    """Build a cached jitted PJRT runner. xt/maskt are replicated (same data
    on every core); weight slices are sharded per core; outputs unsharded on
    host. No donation: the zero output-init buffers stay resident on device
    across calls (the kernel writes every output element)."""
    import jax
    from jax.sharding import Mesh, PartitionSpec, NamedSharding
    try:
        from jax.experimental.shard_map import shard_map
    except ImportError:
        shard_map = jax.shard_map
    from concourse.bass2jax import (_bass_exec_p, install_neuronx_cc_hook,
                                    partition_id_tensor)

    install_neuronx_cc_hook()
    pname = nc.partition_id_tensor.name if nc.partition_id_tensor else None
    in_names, out_names, out_avals, zero_shapes = [], [], [], []
    for alloc in nc.m.functions[0].allocations:
        if not isinstance(alloc, mybir.MemoryLocationSet):
            continue
        name = alloc.memorylocations[0].name
        if alloc.kind == "ExternalInput":
            if name != pname:
                in_names.append(name)
        elif alloc.kind == "ExternalOutput":
            out_names.append(name)
            shape = tuple(alloc.tensor_shape)
            dtype = mybir.dt.np(alloc.dtype)
            out_avals.append(jax.core.ShapedArray(shape, dtype))
            zero_shapes.append((shape, dtype))
    n_params = len(in_names)
    all_in_names = in_names + out_names
    if pname is not None:
        all_in_names = all_in_names + [pname]

    def _body(*args):
        operands = list(args)
        if pname is not None:
            operands.append(partition_id_tensor())
        return tuple(_bass_exec_p.bind(
            *operands,
            out_avals=tuple(out_avals),
            in_names=tuple(all_in_names),
            out_names=tuple(out_names),
            lowering_input_output_aliases=(),
            sim_require_finite=True,
            sim_require_nnan=True,
            nc=nc,
        ))

    devices = jax.devices()[:NCORES]
    mesh = Mesh(np.asarray(devices), ("core",))
    shard = PartitionSpec("core")
    repl = PartitionSpec()
    REPLICATED = ("xt", "maskt")
    in_specs = tuple(repl if n in REPLICATED else shard for n in in_names) \
        + (shard,) * len(out_names)
    sharded = jax.jit(
        shard_map(_body, mesh=mesh, in_specs=in_specs,
                  out_specs=(shard,) * len(out_names), check_rep=False),
        keep_unused=True)
    zeros = [jax.device_put(np.zeros((NCORES * s[0], *s[1:]), d),
                            NamedSharding(mesh, shard))
             for (s, d) in zero_shapes]
    jax.block_until_ready(zeros)

    def run(in_maps):
        args = []
        for n in in_names:
            if n in REPLICATED:
                args.append(jax.device_put(np.asarray(in_maps[0][n]),
                                           NamedSharding(mesh, repl)))
            else:
                args.append(jax.device_put(
                    np.concatenate([np.asarray(m[n]) for m in in_maps], axis=0),
                    NamedSharding(mesh, shard)))
        outs = sharded(*args, *zeros)
        return [
            {n: np.asarray(outs[i]).reshape(NCORES, *out_avals[i].shape)[c]
             for i, n in enumerate(out_names)}
            for c in range(NCORES)
        ]

    return run


def kernel(x, wq, wk, wv, wo):
    x = np.asarray(x, dtype=np.float32)
    wq = np.asarray(wq, dtype=np.float32)
    wk = np.asarray(wk, dtype=np.float32)
    wv = np.asarray(wv, dtype=np.float32)
    wo = np.asarray(wo, dtype=np.float32)

    if "nc" not in _cache:
        _cache["nc"] = _build()
    nc = _cache["nc"]
    in_maps = make_in_maps(x, wq, wk, wv, wo)

    try:
        if "run" not in _cache:
            _cache["run"] = _make_runner(nc)
        results = _cache["run"](in_maps)
    except Exception:
        _cache.pop("run", None)
        results = run_bass_kernel_spmd(
            nc, in_maps, core_ids=list(range(NCORES))).results

    out = np.zeros((SF, D), dtype=np.float64)
    for r in results:
        out += r["outp"].astype(np.float64)
    return out.astype(np.float32).reshape(B, S, D)
